# revision 37
# baseline (speedup 1.0000x reference)
"""Fused multi-head attention block (qkv proj + RoPE + SDPA + out proj) on 8
Trainium2 NeuronCores.

Sharding: data-parallel over batch (4) x tensor-parallel over heads (2 groups
of 8). Core c handles batch c//2, head group c%2. Each core returns a partial
(2048, 1024) output; the host sums the two head-group partials per batch.

All matmul operands are bf16 (fp32 PSUM accumulation). Engine floors per core:
ACT exp stream 256 x [128,1024] tiles ~= 300us, PE ~= 273us (score matmul
pairs co-execute via tile_position quadrants). The kernel keeps the ACT exp
stream gapless:

  - Input DMAs are batched and split across the SP and ACT issue queues
    (descriptor generation serializes per queue at ~0.7-3us per DMA). x^T
    lands in four 512-column pieces; the first piece plus wq/wk/rope tables
    (3.25MB) gates the first score.
  - q/k projection + RoPE run on token windows: the first 512-token window
    feeds SDPA immediately; later windows, the v projection, the second head
    group's q/k, and c_proj are demoted gap-filler for the PE, emitted in
    deadline order.
  - SDPA emission is software-pipelined: for each (head pair, query quarter)
    block and key chunk kc, the attnV matmuls trail the score/exp unit by one
    kc so at block boundaries the next block's scores run under the last exp
    instead of serializing behind attnV. pt (exp output) is 14 deep so attnV
    can lag while the PE pays down the projection debt early on.
  - Softmax normalization reads the y/denominator accumulator directly from
    PSUM (reciprocal + partition_broadcast + multiply), no staging copies,
    keeping the ACT engine exp-only mid-stream.

Per-core layouts:
  xts  [128, 8*T] bf16   x[b].T chunk k at cols [k*T,(k+1)*T)
  q/k produced as [f', t] where f' interleaves the RoPE halves; after RoPE the
      DVE writes head-contiguous chunks: chunk j holds heads (2j, 2j+1), head
      rows [e*64, e*64+64) = [o1(32); o2(32)] -> K=64 contiguous for S^T.
  v   [t, h*65+d] with a ones column per head (col h*65+64) so one matmul
      accumulates y^T and the softmax denominator in a single PSUM group.

Self-contained: hardcodes B=4, T=2048, C=1024, H=16, D=64.
"""

import numpy as np

B, T, C = 4, 2048, 1024
H, D = 16, 64
HL = H // 2            # heads per core
N_CORES = 8
ROPE_THETA = 10000.0

_NC = None


def _build_nc():
    import concourse.mybir as mybir
    import concourse.tile as tile
    from concourse import bacc

    F32 = mybir.dt.float32
    BF16 = mybir.dt.bfloat16
    EXP = mybir.ActivationFunctionType.Exp

    nc = bacc.Bacc("TRN2", target_bir_lowering=False, debug=False, num_devices=N_CORES)

    xt = nc.dram_tensor("xt", [C, T], BF16, kind="ExternalInput")        # x[b].T
    wq = nc.dram_tensor("wq", [C, 512], BF16, kind="ExternalInput")      # [C, f']
    wk = nc.dram_tensor("wk", [C, 512], BF16, kind="ExternalInput")
    wv = nc.dram_tensor("wv", [C, 512], BF16, kind="ExternalInput")      # [C, h*64+d]
    wp = nc.dram_tensor("wp", [512, C], BF16, kind="ExternalInput")      # [h*64+d, o]
    cost = nc.dram_tensor("cost", [128, T], BF16, kind="ExternalInput")
    sint = nc.dram_tensor("sint", [128, T], BF16, kind="ExternalInput")
    eye = nc.dram_tensor("eye", [128, 128], BF16, kind="ExternalInput")
    out = nc.dram_tensor("out", [T, C], BF16, kind="ExternalOutput")

    with tile.TileContext(nc) as tc:
        with (
            tc.tile_pool(name="persist", bufs=1) as pp,
            tc.tile_pool(name="stga", bufs=4) as stga_pool,
            tc.tile_pool(name="stgb", bufs=2) as stgb_pool,
            tc.tile_pool(name="rtmpa", bufs=2) as tmpa_pool,
            tc.tile_pool(name="rtmpb", bufs=2) as tmpb_pool,
            tc.tile_pool(name="ptp", bufs=14) as pt_pool,
            tc.tile_pool(name="obp", bufs=2) as ob_pool,
            tc.tile_pool(name="nrm", bufs=2) as nrm_pool,
            tc.tile_pool(name="pst", bufs=2, space="PSUM") as st_pool,
            tc.tile_pool(name="pya", bufs=2, space="PSUM") as ya_pool,
            tc.tile_pool(name="paux", bufs=2, space="PSUM") as aux_pool,
        ):
            # ---- persistent SBUF tiles -------------------------------------
            xts = pp.tile([128, 8 * T], BF16, name="xts", tag="xts")
            wqs = pp.tile([128, 8 * 512], BF16, name="wqs", tag="wqs")
            wks = pp.tile([128, 8 * 512], BF16, name="wks", tag="wks")
            wvs = pp.tile([128, 8 * 512], BF16, name="wvs", tag="wvs")
            wps = pp.tile([128, 4 * C], BF16, name="wps", tag="wps")
            vbf = [pp.tile([128, 520], BF16, name=f"vbf{t}", tag=f"vbf{t}") for t in range(16)]
            qbf = [pp.tile([128, T], BF16, name=f"qbf{j}", tag=f"qbf{j}") for j in range(4)]
            kbf = [pp.tile([128, T], BF16, name=f"kbf{j}", tag=f"kbf{j}") for j in range(4)]
            ytf = [pp.tile([128, T], BF16, name=f"ytf{c}", tag=f"ytf{c}") for c in range(4)]
            ct = pp.tile([128, T], BF16, name="ct", tag="ct")
            st_ = pp.tile([128, T], BF16, name="st_", tag="st_")
            wrm = pp.tile([128, 512], BF16, name="wrm", tag="wrm")
            eyet = pp.tile([128, 128], BF16, name="eyet", tag="eyet")

            # ---- batched input DMAs, split across SP and ACT issue queues --
            # SP gets the x pieces in column order (its DGE ring throttles the
            # later pieces so the first piece gets the bandwidth); ACT gets the
            # head-critical weights. The late-need DMAs (rope-table tails, wp,
            # eye) are issued after the head section so they never block the
            # ACT queue ahead of the first projection copies.
            def wdma(eng, dst, src, k):
                eng.dma_start(
                    dst[:].rearrange("p (k f) -> p k f", k=k),
                    src[:].rearrange("(k p) f -> p k f", p=128),
                )

            def xdma(lo, hi):
                nc.sync.dma_start(
                    xts[:].rearrange("p (k t) -> p k t", k=8)[:, :, lo:hi],
                    xt[:].rearrange("(k p) t -> p k t", p=128)[:, :, lo:hi],
                )

            def wdma_half(eng, dst, src, k0, k1):
                eng.dma_start(
                    dst[:].rearrange("p (k f) -> p k f", k=8)[:, k0:k1],
                    src[:].rearrange("(k p) f -> p k f", p=128)[:, k0:k1],
                )

            xdma(0, 512)
            wdma_half(nc.scalar, wqs, wq, 0, 4)
            wdma_half(nc.scalar, wqs, wq, 4, 8)
            nc.scalar.dma_start(ct[:, 0:512], cost[:, 0:512])
            nc.scalar.dma_start(st_[:, 0:512], sint[:, 0:512])
            wdma_half(nc.scalar, wks, wk, 0, 4)
            wdma_half(nc.scalar, wks, wk, 4, 8)
            xdma(512, 1024)
            xdma(1024, 1536)
            xdma(1536, 2048)
            wdma(nc.sync, wvs, wv, 8)

            # ---- projection helpers ----------------------------------------
            def proj_mm(ps_ap, wbig, c, lo, hi):
                for k in range(8):
                    nc.tensor.matmul(
                        ps_ap,
                        wbig[:, k * 512 + c * 128: k * 512 + (c + 1) * 128],
                        xts[:, k * T + lo: k * T + hi],
                        start=(k == 0), stop=(k == 7),
                    )

            def rope_half(stage, lo4, half, dst, lo, hi, tpool, nm):
                # o1 = x1*cos - x2*sin ; o2 = x1*sin + x2*cos on window lo:hi,
                # for ONE head pair (stage rows half*64..half*64+64 -> pair
                # j = lo4*2 + half). Shorter DVE chain before the first score.
                # stage: [128, 2*w], chunk c at [0:w), c+2 at [w:2w).
                w_ = hi - lo
                r0 = half * 64
                j = lo4 * 2 + half
                x1 = stage[r0:r0 + 64, 0:w_]
                x2 = stage[r0:r0 + 64, w_:2 * w_]
                cw = ct[r0:r0 + 64, lo:hi]
                sw = st_[r0:r0 + 64, lo:hi]
                a = tpool.tile([64, w_], BF16, name=f"ra_{nm}", tag="tmp")
                nc.vector.tensor_mul(a[:], x1, cw)
                b = tpool.tile([64, w_], BF16, name=f"rb_{nm}", tag="tmp")
                nc.vector.tensor_mul(b[:], x2, sw)
                for e in range(2):
                    nc.vector.tensor_sub(
                        dst[j][e * 64:e * 64 + 32, lo:hi],
                        a[e * 32:e * 32 + 32, :], b[e * 32:e * 32 + 32, :]
                    )
                c2 = tpool.tile([64, w_], BF16, name=f"rc_{nm}", tag="tmp")
                nc.vector.tensor_mul(c2[:], x1, sw)
                d = tpool.tile([64, w_], BF16, name=f"rd_{nm}", tag="tmp")
                nc.vector.tensor_mul(d[:], x2, cw)
                for e in range(2):
                    nc.vector.tensor_add(
                        dst[j][e * 64 + 32:e * 64 + 64, lo:hi],
                        c2[e * 32:e * 32 + 32, :], d[e * 32:e * 32 + 32, :]
                    )

            def rope_pair_w(stage, lo4, dst, lo, hi, tpool, nm):
                rope_half(stage, lo4, 0, dst, lo, hi, tpool, nm + "h0")
                rope_half(stage, lo4, 1, dst, lo, hi, tpool, nm + "h1")

            def head_pair(wbig, c, nm):
                # Chunks (c, c+2) for tokens 0:512 through one [128,1024] PSUM
                # tile (2 banks, free until SDPA starts), one drain copy on the
                # still-idle ACT engine.
                stage = stga_pool.tile([128, 1024], BF16, name=f"stage_{nm}", tag="stg")
                ps = st_pool.tile([128, 1024], F32, name=f"hp_{nm}", tag="st")
                proj_mm(ps[:, 0:512], wbig, c, 0, 512)
                proj_mm(ps[:, 512:1024], wbig, c + 2, 0, 512)
                nc.scalar.copy(stage[:], ps[:])
                return stage

            MED = -500_000

            def piece_pair(wbig, c, lo, w, spool, nm, copy_eng=None):
                # One w-wide token window for chunks (c, c+2) through aux banks.
                # Copies run one tier above the gap-filler matmuls so the aux
                # banks recycle promptly (a stalled queued matmul blocks the
                # whole in-order PE queue behind it).
                stage = spool.tile([128, 2 * w], BF16, name=f"stage_{nm}", tag="stg")
                for ci, cc in ((0, c), (1, c + 2)):
                    for off in range(0, w, 512):
                        ps = aux_pool.tile([128, 512], F32, name=f"pp_{nm}{ci}{off}", tag="aux")
                        proj_mm(ps[:], wbig, cc, lo + off, lo + off + 512)
                        with tc.high_priority(MED):
                            if copy_eng is not None:
                                copy_eng(stage[:, ci * w + off: ci * w + off + 512], ps[:])
                            else:
                                nc.vector.tensor_copy(
                                    stage[:, ci * w + off: ci * w + off + 512], ps[:]
                                )
                return stage

            def v_piece(tm):
                vps = aux_pool.tile([128, 512], F32, name=f"vps{tm}", tag="aux")
                for k in range(8):
                    nc.tensor.matmul(
                        vps[:],
                        xts[:, k * T + tm * 128: k * T + (tm + 1) * 128],
                        wvs[:, k * 512:(k + 1) * 512],
                        start=(k == 0), stop=(k == 7),
                    )
                va = vbf[tm][:].rearrange("p (h x) -> p h x", x=65)
                with tc.high_priority(MED):
                    nc.vector.tensor_copy(
                        va[:, :, 0:64], vps[:].rearrange("p (h d) -> p h d", d=64)
                    )
                    nc.vector.memset(va[:, :, 64], 1.0)

            # ---- SDPA pieces ------------------------------------------------
            def emit_attnv(j, qv, kc, yas, pt_t):
                for e in range(2):
                    h = 2 * j + e
                    nc.tensor.matmul(
                        yas[e][:],
                        vbf[kc][:, h * 65:(h + 1) * 65],
                        pt_t[:, e * 512:(e + 1) * 512],
                        start=(kc == 0), stop=(kc == 15),
                    )

            def emit_norm(j, qv, yas):
                # The y scale reads the accumulator straight from PSUM; the
                # denominator is staged to SBUF for the custom-DVE reciprocal.
                q0 = qv * 512
                for e in range(2):
                    nm2 = f"j{j}v{qv}e{e}"
                    den = nrm_pool.tile([1, 512], F32, name=f"den_{nm2}", tag="den")
                    nc.vector.tensor_copy(den[:], yas[e][64:65, :])
                    rden = nrm_pool.tile([1, 512], F32, name=f"rden_{nm2}", tag="rden")
                    nc.vector.reciprocal_approx_fast(rden[:], den[:])
                    bden = nrm_pool.tile([64, 512], F32, name=f"bden_{nm2}", tag="bden")
                    nc.gpsimd.partition_broadcast(bden[:], rden[:])
                    nc.vector.tensor_mul(
                        ytf[j][e * 64:e * 64 + 64, q0:q0 + 512], yas[e][0:64, :], bden[:]
                    )

            # ---- c_proj for one query quarter (4 qm chunks) -----------------
            def cproj_quarter(qv, pools, tail=False):
                for qm in range(qv * 4, qv * 4 + 4):
                    ob = ob_pool.tile([128, 1024], BF16, name=f"ob{qm}", tag="ob")
                    for oh in range(2):
                        pool, tag = pools[(qm * 2 + oh) % len(pools)]
                        cp = pool.tile([128, 512], F32, name=f"cp{qm}_{oh}", tag=tag)
                        for c in range(4):
                            nc.tensor.matmul(
                                cp[:],
                                ytf[c][:, qm * 128:(qm + 1) * 128],
                                wps[:, c * C + oh * 512: c * C + (oh + 1) * 512],
                                start=(c == 0), stop=(c == 3),
                            )
                        with tc.high_priority(MED):
                            if tail and (qm + oh) % 2 == 0:
                                nc.scalar.copy(ob[:, oh * 512:(oh + 1) * 512], cp[:])
                            else:
                                nc.vector.tensor_copy(ob[:, oh * 512:(oh + 1) * 512], cp[:])
                    nc.sync.dma_start(out[qm * 128:(qm + 1) * 128, :], ob[:])

            # Quarter 3 is split so only one matmul per (qm, oh) remains after
            # the last block: pairs 0-2 are pre-summed into bf16 partials
            # (rounding there is ~0.3% of the partial, well inside tolerance),
            # re-injected into PSUM through an identity matmul at the tail.
            def cproj3_partial(ptiles):
                for qi, qm in enumerate((12, 13, 14, 15)):
                    pt_ = ptiles[qi // 2]
                    pof = (qi % 2) * 1024
                    for oh in range(2):
                        cp = aux_pool.tile([128, 512], F32, name=f"c3p{qm}_{oh}", tag="aux")
                        for c in range(3):
                            nc.tensor.matmul(
                                cp[:],
                                ytf[c][:, qm * 128:(qm + 1) * 128],
                                wps[:, c * C + oh * 512: c * C + (oh + 1) * 512],
                                start=(c == 0), stop=(c == 2),
                            )
                        with tc.high_priority(MED):
                            nc.vector.tensor_copy(
                                pt_[:, pof + oh * 512: pof + (oh + 1) * 512], cp[:]
                            )

            def cproj3_tail(ptiles):
                for qi, qm in enumerate((12, 13, 14, 15)):
                    pt_ = ptiles[qi // 2]
                    pof = (qi % 2) * 1024
                    ob = ob_pool.tile([128, 1024], BF16, name=f"ob{qm}", tag="ob")
                    for oh in range(2):
                        pool = (st_pool, "st") if oh == 0 else (aux_pool, "aux")
                        cp = pool[0].tile([128, 512], F32, name=f"c3t{qm}_{oh}", tag=pool[1])
                        nc.tensor.matmul(
                            cp[:], eyet[:],
                            pt_[:, pof + oh * 512: pof + (oh + 1) * 512],
                            start=True, stop=False,
                        )
                        nc.tensor.matmul(
                            cp[:],
                            ytf[3][:, qm * 128:(qm + 1) * 128],
                            wps[:, 3 * C + oh * 512: 3 * C + (oh + 1) * 512],
                            start=False, stop=True,
                        )
                        if oh == 0:
                            nc.scalar.copy(ob[:, 0:512], cp[:])
                        else:
                            nc.vector.tensor_copy(ob[:, 512:1024], cp[:])
                    nc.sync.dma_start(out[qm * 128:(qm + 1) * 128, :], ob[:])

            # ---- program order (dataflow) + scheduler priorities ------------
            LOW = -1_000_000

            # PE p-state warmers: the PE idles ~10us waiting for the first x
            # piece and would start the projections at the low clock; these
            # chained dummy matmuls keep it ramped (LOW: they yield to real
            # work the moment it is ready).
            with tc.high_priority(LOW):
                nc.vector.memset(wrm[:], 0.0)
                wps_warm = aux_pool.tile([128, 512], F32, name="warm", tag="aux")
                for i in range(24):
                    nc.tensor.matmul(
                        wps_warm[:], wrm[:, 0:128], wrm[:], start=True, stop=True
                    )

            sA = head_pair(wqs, 0, "qA")
            rope_pair_w(sA, 0, qbf, 0, 512, tmpa_pool, "qA")
            sA = head_pair(wks, 0, "kA")
            rope_pair_w(sA, 0, kbf, 0, 512, tmpa_pool, "kA")
            # late-need DMA issues, queued behind the head's ACT copies
            nc.scalar.dma_start(ct[:, 512:T], cost[:, 512:T])
            nc.scalar.dma_start(st_[:, 512:T], sint[:, 512:T])
            wdma(nc.scalar, wps, wp, 4)
            nc.scalar.dma_start(eyet[:], eye[:])

            # Gap-filler tier, emitted in deadline order: k windows feed the
            # running block's kc loop (their drain copies ride the still-idle
            # ACT engine), v feeds its (lagging) attnV, q windows feed qv>=1,
            # lo1 feeds pairs 2/3.
            with tc.high_priority(LOW):
                s = piece_pair(wks, 0, 512, 512, stga_pool, "kB1", copy_eng=nc.scalar.copy)
                rope_pair_w(s, 0, kbf, 512, 1024, tmpa_pool, "kB1")
                s = piece_pair(wks, 0, 1024, 512, stga_pool, "kB2", copy_eng=nc.scalar.copy)
                rope_pair_w(s, 0, kbf, 1024, 1536, tmpa_pool, "kB2")
                s = piece_pair(wks, 0, 1536, 512, stga_pool, "kB3")
                rope_pair_w(s, 0, kbf, 1536, 2048, tmpa_pool, "kB3")
                # qB windows gate scores directly (no runway), so qBa comes
                # before v (whose attnV consumers ride the pt runway).
                s = piece_pair(wqs, 0, 512, 1024, stgb_pool, "qBa")
                rope_pair_w(s, 0, qbf, 512, 1536, tmpb_pool, "qBa")
                for tm in range(8):
                    v_piece(tm)
                s = piece_pair(wqs, 0, 1536, 512, stga_pool, "qBb")
                rope_pair_w(s, 0, qbf, 1536, 2048, tmpa_pool, "qBb")
                for tm in range(8, 16):
                    v_piece(tm)
                # lo1: chunks (1,3) -> pairs 2,3, in two 1024 windows each
                for lo in (0, 1024):
                    s = piece_pair(wks, 1, lo, 1024, stgb_pool, f"k1_{lo}")
                    rope_pair_w(s, 1, kbf, lo, lo + 1024, tmpb_pool, f"k1_{lo}")
                for lo in (0, 1024):
                    s = piece_pair(wqs, 1, lo, 1024, stgb_pool, f"q1_{lo}")
                    rope_pair_w(s, 1, qbf, lo, lo + 1024, tmpb_pool, f"q1_{lo}")

            # ---- SDPA main loop: classic per-block emission ------------------
            def sdpa_block(j, qv):
                q0 = qv * 512
                yas = [
                    ya_pool.tile([65, 512], F32, name=f"ya_j{j}v{qv}e{e}", tag="ya")
                    for e in range(2)
                ]
                for kc in range(16):
                    stt = st_pool.tile(
                        [128, 1024], F32, name=f"st_j{j}v{qv}k{kc}", tag="st"
                    )
                    for e in range(2):
                        nc.tensor.matmul(
                            stt[:, e * 512:(e + 1) * 512],
                            kbf[j][e * 64:e * 64 + 64, kc * 128:(kc + 1) * 128],
                            qbf[j][e * 64:e * 64 + 64, q0:q0 + 512],
                            start=True, stop=True,
                            tile_position=(e * 64, 0),
                        )
                    pt_t = pt_pool.tile(
                        [128, 1024], BF16, name=f"pt_j{j}v{qv}k{kc}", tag="pt"
                    )
                    nc.scalar.activation(pt_t[:], stt[:], EXP, scale=0.125)
                    emit_attnv(j, qv, kc, yas, pt_t)
                emit_norm(j, qv, yas)

            sdpa_block(0, 0)
            sdpa_block(1, 0)
            sdpa_block(0, 1)
            sdpa_block(1, 1)
            sdpa_block(0, 2)
            sdpa_block(1, 2)
            sdpa_block(0, 3)
            sdpa_block(1, 3)
            sdpa_block(2, 0)
            sdpa_block(3, 0)
            with tc.high_priority(LOW):
                cproj_quarter(0, [(aux_pool, "aux")])
            sdpa_block(2, 1)
            sdpa_block(3, 1)
            with tc.high_priority(LOW):
                cproj_quarter(1, [(aux_pool, "aux")])
            sdpa_block(2, 2)
            sdpa_block(3, 2)
            with tc.high_priority(LOW):
                cproj_quarter(2, [(aux_pool, "aux")])
            sdpa_block(2, 3)
            c3p = [
                stgb_pool.tile([128, 2048], BF16, name=f"c3p{i}", tag="stg")
                for i in range(2)
            ]
            with tc.high_priority(LOW):
                cproj3_partial(c3p)
            sdpa_block(3, 3)
            cproj3_tail(c3p)

    nc.compile()
    return nc


def _qk_perm():
    """f' (0..511) -> within-group feature index (h*64 + d) for q/k.

    f' = half*256 + (h//4)*128 + (h%4)*32 + i maps to d = 2*i + half.
    """
    perm = np.zeros(512, dtype=np.int64)
    for h in range(HL):
        for i in range(32):
            perm[(h // 4) * 128 + (h % 4) * 32 + i] = h * 64 + 2 * i
            perm[256 + (h // 4) * 128 + (h % 4) * 32 + i] = h * 64 + 2 * i + 1
    return perm


def _rope_tables():
    import ml_dtypes

    i = np.arange(128) % 32
    inv = (1.0 / (ROPE_THETA ** (np.arange(0, D, 2, dtype=np.float32) / D))).astype(np.float32)
    ang = np.arange(T, dtype=np.float32)[None, :] * inv[i][:, None]
    return (
        np.cos(ang).astype(ml_dtypes.bfloat16),
        np.sin(ang).astype(ml_dtypes.bfloat16),
    )


def make_in_maps(x, w_attn, w_proj):
    import ml_dtypes

    bf = ml_dtypes.bfloat16
    x = np.asarray(x, dtype=np.float32)
    w_attn = np.asarray(w_attn, dtype=np.float32)
    w_proj = np.asarray(w_proj, dtype=np.float32)
    perm = _qk_perm()
    cost, sint = _rope_tables()
    eye = np.eye(128, dtype=bf)
    in_maps = []
    xts = [np.ascontiguousarray(x[b].T.astype(bf)) for b in range(B)]
    for core in range(N_CORES):
        b, g = core // 2, core % 2
        base = g * 512
        wqc = np.ascontiguousarray(w_attn[base + perm, :].T.astype(bf))
        wkc = np.ascontiguousarray(w_attn[C + base + perm, :].T.astype(bf))
        wvc = np.ascontiguousarray(w_attn[2 * C + base:2 * C + base + 512, :].T.astype(bf))
        wpc = np.ascontiguousarray(w_proj[:, base:base + 512].T.astype(bf))
        in_maps.append(
            {"xt": xts[b], "wq": wqc, "wk": wkc, "wv": wvc, "wp": wpc,
             "cost": cost, "sint": sint, "eye": eye}
        )
    return in_maps


def kernel(x, w_attn, w_proj):
    global _NC
    from concourse.bass_utils import run_bass_kernel_spmd

    if _NC is None:
        _NC = _build_nc()
    in_maps = make_in_maps(x, w_attn, w_proj)
    res = run_bass_kernel_spmd(_NC, in_maps, list(range(N_CORES))).results
    out = np.empty((B, T, C), dtype=np.float32)
    for b in range(B):
        out[b] = res[2 * b]["out"].astype(np.float32) + res[2 * b + 1]["out"].astype(
            np.float32
        )
    return out
